# revision 9
# baseline (speedup 1.0000x reference)
"""Trainium2 Bass kernel for BondEmbedding (GNN edge embedding).

out[e, :] = concat(bond_feat[e], gaussian_smearing(|pos[i0[e]] - pos[i1[e]]|)) @ W + b

Sharding: edges split across 8 NeuronCores (embarrassingly parallel);
pos table / weights / constants replicated on every core.

Per-core dataflow (supertile = 4096 edges, K=32 edges per SBUF partition):
  - HWDGE DMA: bond_feat slab, block-idx / remainder slabs, output slab
  - SWDGE dma_gather: the pos table is packed as [25000, 64] f32 (4 nodes
    per 256B block, each node a 64B row [x,y,z,0,...]); per edge endpoint
    we gather the 256B block containing the node (block index fits int16,
    which dma_gather requires), then select the node's 16B row on DVE via
    a 4-wide one-hot (built from idx%4) and a grouped reduce
  - ACT: d = exp(0.5*ln(dist2)) (one table set: natural_log_exp_and_others
    covers Ln/Exp/Copy -> single table load, no sqrt-ULP hazard)
  - DVE/ACT: gauss features exp(coeff*(d-offset)^2) written into a packed
    [128, K*84] feature tile next to the bond features
  - PE: per 128-edge chunk, transpose feat [128,84] -> [84,128] (via
    identity), then matmul(lhsT=featT, rhs=W[84,128]) -> psum [128e,128o]
  - DVE: psum + bias -> SBUF, one big store DMA per supertile

dma_gather quirks handled here: indices live in partitions 0-15 wrapped
(i%16, i//16) and must be replicated to all 8 partition groups; output is
partition-fastest (gather position i -> partition i%128, slot i//128), so
the host feeds indices in transposed order to land edge (p,k)=e0+p*K+k at
[p, k]; single_packet=True wedges the SDMA (device unrecoverable) so we
always pass single_packet=False.
"""

import sys

sys.path.insert(0, "/opt/trn_rl_repo")

import numpy as np

E_TOTAL = 2_000_000
N_NODES = 100_000
IN_DIM = 64
OUT_DIM = 128
NG = 20
CUTOFF = 10.0
FEAT = IN_DIM + NG  # 84
N_BLOCKS = N_NODES // 4  # 25000 blocks of 4 nodes (256B each)

N_CORES = 8
SHARD = E_TOTAL // N_CORES  # 250000
K = 32                      # edges per partition per supertile
S = 128 * K                 # 4096 edges per supertile
NT = 62                     # supertiles per core
E_PC = S * NT               # 253952 edges processed per core (overlapped shards)

_DELTA = CUTOFF / (NG - 1)
COEFF = -0.5 / (_DELTA * _DELTA)

_prog_cache = {}
BIAS_OP = "add"  # test hook: "sub" flips the bias op to probe NEFF caching
WORK_BUFS = 3   # work-pool buffering
PO_BATCH = 4    # matmuls accumulated per psum-out tile (4 -> 1 bank, 8 -> 2)


def build_program(e_pc, nt, k, repeat=1):
    """Build the per-core Bass program (identical on all cores).

    repeat>1 re-runs the whole edge sweep that many times (same inputs and
    outputs) — used only for slope-based wall-clock timing.
    """
    from concourse import bacc, mybir, tile
    from concourse.masks import make_identity

    f32 = mybir.dt.float32
    i16 = mybir.dt.int16
    ALU = mybir.AluOpType
    ACT = mybir.ActivationFunctionType

    s = 128 * k
    nw = s // 16  # wrapped idx columns per supertile

    nc = bacc.Bacc("TRN2", target_bir_lowering=False, debug=False)

    bond = nc.dram_tensor("bond_feat", [e_pc, IN_DIM], f32, kind="ExternalInput")
    blk0 = nc.dram_tensor("blk0", [nt, 128, nw], i16, kind="ExternalInput")
    blk1 = nc.dram_tensor("blk1", [nt, 128, nw], i16, kind="ExternalInput")
    rem0 = nc.dram_tensor("rem0", [nt, 128, k], f32, kind="ExternalInput")
    rem1 = nc.dram_tensor("rem1", [nt, 128, k], f32, kind="ExternalInput")
    tab = nc.dram_tensor("tab", [N_BLOCKS, 64], f32, kind="ExternalInput")
    wt = nc.dram_tensor("w", [FEAT, OUT_DIM], f32, kind="ExternalInput")
    offs = nc.dram_tensor("offs", [128, NG], f32, kind="ExternalInput")
    bias = nc.dram_tensor("bias", [128, OUT_DIM], f32, kind="ExternalInput")
    cand = nc.dram_tensor("cand", [128, 4], f32, kind="ExternalInput")
    out = nc.dram_tensor("out", [e_pc, OUT_DIM], f32, kind="ExternalOutput")

    with tile.TileContext(nc) as tc:
        with (
            tc.tile_pool(name="const", bufs=1) as cpool,
            tc.tile_pool(name="work", bufs=WORK_BUFS) as pool,
            tc.tile_pool(name="psum", bufs=2, space="PSUM") as ppool,
        ):
            w_sb = cpool.tile([FEAT, OUT_DIM], f32, tag="w")
            nc.sync.dma_start(out=w_sb[:], in_=wt[:, :])
            offs_sb = cpool.tile([128, NG], f32, tag="offs")
            nc.sync.dma_start(out=offs_sb[:], in_=offs[:, :])
            bias_sb = cpool.tile([128, OUT_DIM], f32, tag="bias")
            nc.sync.dma_start(out=bias_sb[:], in_=bias[:, :])
            cand_sb = cpool.tile([128, 4], f32, tag="cand")
            nc.sync.dma_start(out=cand_sb[:], in_=cand[:, :])
            ident = cpool.tile([128, 128], f32, tag="ident")
            make_identity(nc, ident[:])

            for t in range(nt * repeat):
                t = t % nt
                e0 = t * s

                # --- gather both endpoints' pos blocks, select rows -------
                sel = []
                for r, (blkd, remd) in enumerate(((blk0, rem0), (blk1, rem1))):
                    bt = pool.tile([128, nw], i16, tag=f"blk{r}")
                    nc.sync.dma_start(out=bt[:], in_=blkd[t, :, :])
                    gth = pool.tile([128, (s // 128) * 64], f32, tag=f"gth{r}")
                    nc.gpsimd.dma_gather(
                        out_ap=gth[:].rearrange("p (k c) -> p k c", c=64),
                        in_ap=tab[:, :],
                        idxs_ap=bt[:],
                        num_idxs=s,
                        num_idxs_reg=s,
                        elem_size=64,
                        single_packet=False,
                    )
                    rt = pool.tile([128, k], f32, tag=f"rem{r}")
                    nc.sync.dma_start(out=rt[:], in_=remd[t, :, :])
                    oh = pool.tile([128, 4 * k], f32, tag=f"oh{r}")
                    nc.vector.tensor_tensor(
                        out=oh[:].rearrange("p (k m) -> p k m", m=4),
                        in0=rt[:].unsqueeze(2).to_broadcast([128, k, 4]),
                        in1=cand_sb[:].unsqueeze(1).to_broadcast([128, k, 4]),
                        op=ALU.is_equal,
                    )
                    tmp = pool.tile([128, 16 * k], f32, tag=f"tmp{r}")
                    # gth element (k, m, v): edge chunk k, node-slot m, 16-f32
                    # row v; want [p][k][c=v<4][m] ordering with m innermost
                    gv = gth[:].rearrange("p (k m v) -> p k v m", m=4, v=16)
                    nc.vector.tensor_tensor(
                        out=tmp[:].rearrange("p (k c m) -> p k c m", c=4, m=4),
                        in0=gv[:, :, 0:4, :],
                        in1=oh[:]
                        .rearrange("p (k m) -> p k m", m=4)
                        .unsqueeze(2)
                        .to_broadcast([128, k, 4, 4]),
                        op=ALU.mult,
                    )
                    rr = pool.tile([128, 4 * k], f32, tag=f"r{r}")
                    nc.vector.tensor_reduce(
                        out=rr[:].rearrange("p (k c) -> p k c", c=4),
                        in_=tmp[:].rearrange("p (k c m) -> p k c m", c=4, m=4),
                        axis=mybir.AxisListType.X,
                        op=ALU.add,
                    )
                    sel.append(rr)

                # --- distance -> d = sqrt(dist2) via exp(0.5*ln(.)) -------
                diff = pool.tile([128, 4 * k], f32, tag="diff")
                nc.vector.tensor_tensor(
                    out=diff[:], in0=sel[0][:], in1=sel[1][:], op=ALU.subtract
                )
                sq = pool.tile([128, 4 * k], f32, tag="sq")
                nc.vector.tensor_tensor(out=sq[:], in0=diff[:], in1=diff[:], op=ALU.mult)
                dist2 = pool.tile([128, k], f32, tag="dist2")
                nc.vector.tensor_reduce(
                    out=dist2[:],
                    in_=sq[:].rearrange("p (k c) -> p k c", c=4),
                    axis=mybir.AxisListType.X,
                    op=ALU.add,
                )
                # clamp so ln() stays finite; exp(0.5*ln(1e-35)) ~ 3e-18 ~ 0
                nc.vector.tensor_scalar_max(out=dist2[:], in0=dist2[:], scalar1=1e-35)
                d = pool.tile([128, k], f32, tag="d")
                nc.scalar.activation(out=d[:], in_=dist2[:], func=ACT.Ln)
                nc.scalar.activation(out=d[:], in_=d[:], func=ACT.Exp, scale=0.5)

                # --- feature tile [128, k*84] ------------------------------
                feat = pool.tile([128, FEAT * k], f32, tag="feat")
                featv = feat[:].rearrange("p (k f) -> p k f", f=FEAT)

                bf = pool.tile([128, IN_DIM * k], f32, tag="bf")
                nc.sync.dma_start(
                    out=bf[:],
                    in_=bond[e0 : e0 + s, :].rearrange("(p k) f -> p (k f)", p=128),
                )
                nc.scalar.activation(
                    out=featv[:, :, 0:IN_DIM],
                    in_=bf[:].rearrange("p (k f) -> p k f", f=IN_DIM),
                    func=ACT.Copy,
                )

                u = pool.tile([128, NG * k], f32, tag="u")
                uv = u[:].rearrange("p (k g) -> p k g", g=NG)
                nc.vector.tensor_tensor(
                    out=uv,
                    in0=d[:].unsqueeze(2).to_broadcast([128, k, NG]),
                    in1=offs_sb[:].unsqueeze(1).to_broadcast([128, k, NG]),
                    op=ALU.subtract,
                )
                gslice = featv[:, :, IN_DIM:FEAT]
                nc.vector.tensor_tensor(out=gslice, in0=uv, in1=uv, op=ALU.mult)
                nc.scalar.activation(out=gslice, in_=gslice, func=ACT.Exp, scale=COEFF)

                # --- transpose + matmul + bias -----------------------------
                outsb = pool.tile([128, OUT_DIM * k], f32, tag="outsb")
                outv = outsb[:].rearrange("p (k o) -> p k o", o=OUT_DIM)

                po = None
                for j in range(k // 8):
                    pt = ppool.tile([FEAT, 8 * 128], f32, tag="pt")
                    for i in range(8):
                        kk = 8 * j + i
                        nc.tensor.transpose(
                            out=pt[:, 128 * i : 128 * (i + 1)],
                            in_=feat[:, FEAT * kk : FEAT * (kk + 1)],
                            identity=ident[:],
                        )
                    ft = pool.tile([FEAT, 8 * 128], f32, tag="ft")
                    nc.scalar.activation(out=ft[:], in_=pt[:], func=ACT.Copy)
                    pb = PO_BATCH
                    for i in range(8):
                        kk = 8 * j + i
                        m = kk % pb
                        if m == 0:
                            po = ppool.tile([128, 128 * pb], f32, tag="po")
                        nc.tensor.matmul(
                            out=po[:, 128 * m : 128 * (m + 1)],
                            lhsT=ft[:, 128 * i : 128 * (i + 1)],
                            rhs=w_sb[:],
                            start=True,
                            stop=True,
                        )
                        if m == pb - 1:
                            nc.vector.tensor_tensor(
                                out=outv[:, kk - pb + 1 : kk + 1, :],
                                in0=po[:].rearrange("p (q o) -> p q o", o=OUT_DIM),
                                in1=bias_sb[:].unsqueeze(1).to_broadcast(
                                    [128, pb, OUT_DIM]
                                ),
                                op=ALU.add if BIAS_OP == "add" else ALU.subtract,
                            )

                nc.sync.dma_start(
                    out=out[e0 : e0 + s, :].rearrange("(p k) o -> p (k o)", p=128),
                    in_=outsb[:],
                )

    nc.compile()
    return nc


def get_program(e_pc=E_PC, nt=NT, k=K):
    key = (e_pc, nt, k)
    if key not in _prog_cache:
        _prog_cache[key] = build_program(e_pc, nt, k)
    return _prog_cache[key]


def _gather_inputs(idx, nt, k):
    """blk (wrapped+replicated int16 block idx) and rem (f32 idx%4) slabs."""
    s = 128 * k
    nw = s // 16
    # gather-position i covers local edge (i%128)*k + i//128
    ii = np.arange(s)
    perm = (ii % 128) * k + (ii // 128)
    blk = (idx >> 2).astype(np.int16).reshape(nt, s)[:, perm]  # [nt, s]
    wrapped = blk.reshape(nt, nw, 16).transpose(0, 2, 1)  # [nt, 16, nw]
    blk_t = np.broadcast_to(wrapped[:, None, :, :], (nt, 8, 16, nw)).reshape(
        nt, 128, nw
    )
    rem = (idx & 3).astype(np.float32).reshape(nt, 128, k)
    return np.ascontiguousarray(blk_t), np.ascontiguousarray(rem)


def make_in_maps(bond_feat, bond_index, pos_nodes, W, b, e_pc=E_PC, nt=NT, k=K):
    """Shard the full problem into per-core input maps.

    Core c handles edges [c*SHARD, c*SHARD + e_pc) (wrapping around at
    E_TOTAL); rows beyond the first SHARD are redundant overlap so every
    core runs the identical static program.
    """
    bond_feat = np.ascontiguousarray(bond_feat, dtype=np.float32)
    idx0_all = np.ascontiguousarray(bond_index[0], dtype=np.int32)
    idx1_all = np.ascontiguousarray(bond_index[1], dtype=np.int32)

    tab = np.zeros((N_NODES, 16), dtype=np.float32)
    tab[:, :3] = pos_nodes
    tab = tab.reshape(N_BLOCKS, 64)

    offs_row = np.linspace(0.0, CUTOFF, NG, dtype=np.float32)
    offs_bcast = np.ascontiguousarray(np.broadcast_to(offs_row, (128, NG)))
    bias_bcast = np.ascontiguousarray(
        np.broadcast_to(np.asarray(b, dtype=np.float32), (128, OUT_DIM))
    )
    cand = np.ascontiguousarray(
        np.broadcast_to(np.arange(4, dtype=np.float32), (128, 4))
    )
    w_np = np.ascontiguousarray(W, dtype=np.float32)

    def wrap_slice(arr, start, n):
        end = start + n
        if end <= E_TOTAL:
            return arr[start:end]
        return np.concatenate([arr[start:], arr[: end - E_TOTAL]], axis=0)

    in_maps = []
    for c in range(N_CORES):
        start = c * SHARD
        i0 = wrap_slice(idx0_all, start, e_pc)
        i1 = wrap_slice(idx1_all, start, e_pc)
        b0, r0 = _gather_inputs(i0, nt, k)
        b1, r1 = _gather_inputs(i1, nt, k)
        in_maps.append(
            {
                "bond_feat": wrap_slice(bond_feat, start, e_pc),
                "blk0": b0,
                "blk1": b1,
                "rem0": r0,
                "rem1": r1,
                "tab": tab,
                "w": w_np,
                "offs": offs_bcast,
                "bias": bias_bcast,
                "cand": cand,
            }
        )
    return in_maps


def kernel(bond_feat, bond_index, pos_nodes, W, b):
    from concourse.bass_utils import run_bass_kernel_spmd

    nc = get_program()
    in_maps = make_in_maps(bond_feat, bond_index, pos_nodes, W, b)
    res = run_bass_kernel_spmd(nc, in_maps, core_ids=list(range(N_CORES)))

    full = np.empty((E_TOTAL, OUT_DIM), dtype=np.float32)
    for c in range(N_CORES):
        full[c * SHARD : (c + 1) * SHARD] = res.results[c]["out"][:SHARD]
    return full


def reference_numpy(bond_feat, bond_index, pos_nodes, W, b):
    """Pure-numpy oracle for local testing."""
    diff = pos_nodes[bond_index[0]] - pos_nodes[bond_index[1]]
    dist = np.sqrt(np.sum(diff * diff, axis=-1))
    offs_row = np.linspace(0.0, CUTOFF, NG, dtype=np.float32)
    dd = dist[:, None] - offs_row[None, :]
    gauss = np.exp(COEFF * dd * dd)
    feat = np.concatenate([bond_feat, gauss.astype(np.float32)], axis=-1)
    return feat @ W + b
